# revision 10
# baseline (speedup 1.0000x reference)
"""DVAE GNN message-passing kernel for 8 Trainium2 NeuronCores.

Math restructuring (the core trick):
  X = one_hot(node_types, 32) is one-hot over only NVT=32 types, so every
  per-type linear layer is a 32-row table lookup, and the predecessor
  aggregation factors through a tiny "predecessor type count" matrix:

     C[b]   = A[b]^T @ X[b]                  [256, 32]  (ints)
     Hpre   = C @ T_gate                     T_gate = sigmoid(Wg^T+bg)*Wm^T  [32,128]
     gi     = X @ W_ih^T + b_ih              table T_ih = W_ih^T             [32,384]
     gh     = Hpre @ W_hh^T + b_hh = C @ T_gh + b_hh,  T_gh = T_gate @ W_hh^T [32,384]

  Per batch the only O(N^2) work is C^T = X^T A (two 128-contraction
  matmuls, exact in bf16 since everything is 0/1 and counts < 256); all
  other matmuls are K=32 table matmuls done in fp32r at full PE rate.
  GRU elementwise + graph-sum are fused DVE/ACT ops on [128, 254] tiles
  in (feature, node) layout; only nodes 1..254 are ever consumed.

Sharding: pure data parallel, 64 batches per core; tables replicated.
"""

import sys
from contextlib import ExitStack

import numpy as np

sys.path.insert(0, "/opt/trn_rl_repo")

import ml_dtypes  # noqa: E402
import concourse.bass as bass  # noqa: E402
import concourse.bacc as bacc  # noqa: E402
import concourse.tile as tile  # noqa: E402
from concourse import mybir  # noqa: E402
from concourse.bass_utils import run_bass_kernel_spmd  # noqa: E402

B, N, NVT, HS, NZ = 512, 256, 32, 128, 64
NCORES = 8
BL = B // NCORES  # 64 batches per core

F32 = mybir.dt.float32
F32R = mybir.dt.float32r
BF16 = mybir.dt.bfloat16
EQ = mybir.AluOpType.is_equal
ADD = mybir.AluOpType.add
SUB = mybir.AluOpType.subtract
MUL = mybir.AluOpType.mult
SIG = mybir.ActivationFunctionType.Sigmoid
TANH = mybir.ActivationFunctionType.Tanh
IDENT = mybir.ActivationFunctionType.Identity

# layout of the packed f32 constant block [128, CW]
C_IOTA32 = 0          # [128, 32]
C_NTCOLS = 32         # [128, 2*BL]
C_BIAS4 = 32 + 2 * BL         # [128, 4]
C_W12 = C_BIAS4 + 4           # [128, 2*NZ]
C_B12 = C_W12 + 2 * NZ        # [64, 2]
C_IOTAP = C_B12 + 2           # [32, 1]
CW = C_IOTAP + 1

_CACHE = {}


def _build_nc():
    nc = bacc.Bacc()

    a_d = nc.declare_dram_parameter("a_t", [BL, 128, 512], BF16, False)
    cst_d = nc.declare_dram_parameter("cst", [128, CW], F32, False)
    tbl_d = nc.declare_dram_parameter("tbl", [32, 896], F32R, False)
    ntb_d = nc.declare_dram_parameter("nt_bcast", [32, BL * N], F32, False)
    out_d = nc.declare_dram_parameter("out", [NZ, 2 * BL], F32, True)

    with tile.TileContext(nc) as tc, ExitStack() as ctx:
        cpool = ctx.enter_context(tc.tile_pool(name="const", bufs=1))
        apool = ctx.enter_context(tc.tile_pool(name="a", bufs=3))
        xpool = ctx.enter_context(tc.tile_pool(name="x", bufs=3))
        epool = ctx.enter_context(tc.tile_pool(name="ew", bufs=3))
        ppool = ctx.enter_context(tc.tile_pool(name="ps", bufs=2, space="PSUM"))

        cst = cpool.tile([128, CW], F32, tag="cst")
        nc.sync.dma_start(out=cst, in_=cst_d[:, :])
        tbl = cpool.tile([32, 896], F32R, tag="tbl")
        nc.sync.dma_start(out=tbl, in_=tbl_d[:, :])
        nt_bcast = cpool.tile([32, BL * N], F32, tag="ntb")
        nc.sync.dma_start(out=nt_bcast, in_=ntb_d[:, :])

        iota32 = cst[:, C_IOTA32 : C_IOTA32 + 32]
        nt_cols = cst[:, C_NTCOLS : C_NTCOLS + 2 * BL]
        bias4 = cst[:, C_BIAS4 : C_BIAS4 + 4]
        w12 = cst[:, C_W12 : C_W12 + 2 * NZ]
        b12 = cst[0:NZ, C_B12 : C_B12 + 2]
        iotap = cst[0:32, C_IOTAP : C_IOTAP + 1]
        t_ih = tbl[:, 0:384]
        t_gh = tbl[:, 384:768]
        t_gate = tbl[:, 768:896]

        hg = cpool.tile([128, BL], F32, tag="hg")
        out_sb = cpool.tile([NZ, 2 * BL], F32, tag="out_sb")

        for b in range(BL):
            a_t = apool.tile([128, 512], BF16, tag="a")
            nc.sync.dma_start(out=a_t, in_=a_d[b, :, :])

            # one-hot of this batch's node types, in both layouts
            x01 = xpool.tile([128, 64], BF16, tag="x01")
            nc.gpsimd.tensor_scalar(
                out=x01[:, 0:32], in0=iota32, scalar1=nt_cols[:, 2 * b : 2 * b + 1],
                scalar2=None, op0=EQ)
            nc.gpsimd.tensor_scalar(
                out=x01[:, 32:64], in0=iota32, scalar1=nt_cols[:, 2 * b + 1 : 2 * b + 2],
                scalar2=None, op0=EQ)
            xt = xpool.tile([32, N], F32R, tag="xt")
            nc.gpsimd.tensor_scalar(
                out=xt, in0=nt_bcast[:, b * N : (b + 1) * N], scalar1=iotap,
                scalar2=None, op0=EQ)

            # C^T[t, v] = sum_u X[u, t] A[u, v]   (exact integer counts)
            c_ps = ppool.tile([32, N], F32, tag="c")
            nc.tensor.matmul(out=c_ps, lhsT=x01[:, 0:32], rhs=a_t[:, 0:256],
                             start=True, stop=False)
            nc.tensor.matmul(out=c_ps, lhsT=x01[:, 32:64], rhs=a_t[:, 256:512],
                             start=False, stop=True)
            c_sb = xpool.tile([32, N], F32R, tag="csb")
            nc.vector.tensor_copy(out=c_sb, in_=c_ps)

            # gate pre-activations: [gate_chunk, node] layout
            rz_ps = ppool.tile([128, 512], F32, tag="rz")
            nc.tensor.matmul(out=rz_ps[:, 0:256], lhsT=t_ih[:, 0:128],
                             rhs=xt, start=True, stop=False)
            nc.tensor.matmul(out=rz_ps[:, 0:256], lhsT=t_gh[:, 0:128],
                             rhs=c_sb, start=False, stop=True)
            nc.tensor.matmul(out=rz_ps[:, 256:512], lhsT=t_ih[:, 128:256],
                             rhs=xt, start=True, stop=False)
            nc.tensor.matmul(out=rz_ps[:, 256:512], lhsT=t_gh[:, 128:256],
                             rhs=c_sb, start=False, stop=True)
            n_ps = ppool.tile([128, 512], F32, tag="n")
            nc.tensor.matmul(out=n_ps[:, 0:256], lhsT=t_ih[:, 256:384],
                             rhs=xt)
            nc.tensor.matmul(out=n_ps[:, 256:512], lhsT=t_gh[:, 256:384],
                             rhs=c_sb)
            hp_ps = ppool.tile([128, N], F32, tag="hp")
            nc.tensor.matmul(out=hp_ps, lhsT=t_gate, rhs=c_sb)

            # GRU + graph-sum on nodes 1..254 only
            r_t = epool.tile([128, 254], F32, tag="r")
            nc.scalar.activation(out=r_t, in_=rz_ps[:, 1:255], func=SIG,
                                 bias=bias4[:, 0:1], scale=1.0)
            z_t = epool.tile([128, 254], F32, tag="z")
            nc.scalar.activation(out=z_t, in_=rz_ps[:, 257:511], func=SIG,
                                 bias=bias4[:, 1:2], scale=1.0)
            t_t = epool.tile([128, 254], F32, tag="t")
            nc.vector.scalar_tensor_tensor(
                out=t_t, in0=n_ps[:, 257:511], scalar=bias4[:, 3:4], in1=r_t,
                op0=ADD, op1=MUL)
            s_t = epool.tile([128, 254], F32, tag="s")
            nc.vector.tensor_tensor(out=s_t, in0=t_t, in1=n_ps[:, 1:255], op=ADD)
            nn_t = epool.tile([128, 254], F32, tag="nn")
            nc.scalar.activation(out=nn_t, in_=s_t, func=TANH,
                                 bias=bias4[:, 2:3], scale=1.0)
            d_t = epool.tile([128, 254], F32, tag="d")
            nc.vector.tensor_tensor(out=d_t, in0=hp_ps[:, 1:255], in1=nn_t, op=SUB)
            e_t = epool.tile([128, 254], F32, tag="e")
            nc.vector.tensor_tensor(out=e_t, in0=z_t, in1=d_t, op=MUL)
            f_t = epool.tile([128, 254], F32, tag="f")
            nc.vector.scalar_tensor_tensor(
                out=f_t, in0=nn_t, scalar=0.0, in1=e_t, op0=ADD, op1=ADD,
                accum_out=hg[:, b : b + 1])

        # mu^T / logvar^T: [z, batch] = W^T.T @ Hg_all
        mu_ps = ppool.tile([NZ, BL], F32, tag="c")
        nc.tensor.matmul(out=mu_ps, lhsT=w12[:, 0:NZ], rhs=hg)
        nc.scalar.activation(out=out_sb[:, 0:BL], in_=mu_ps, func=IDENT,
                             bias=b12[:, 0:1], scale=1.0)
        lv_ps = ppool.tile([NZ, BL], F32, tag="c")
        nc.tensor.matmul(out=lv_ps, lhsT=w12[:, NZ : 2 * NZ], rhs=hg)
        nc.scalar.activation(out=out_sb[:, BL : 2 * BL], in_=lv_ps, func=IDENT,
                             bias=b12[:, 1:2], scale=1.0)
        nc.sync.dma_start(out=out_d[:, :], in_=out_sb)

    nc.finalize()
    return nc


def kernel(node_types, adj, W_ih, W_hh, b_ih, b_hh, Wg, bg, Wm, W1, b1, W2, b2):
    bf16 = ml_dtypes.bfloat16

    # host-folded tables (weight preprocessing only; all O(B) work on device)
    t_gate = (1.0 / (1.0 + np.exp(-(Wg.T.astype(np.float64) + bg[None, :])))) * Wm.T
    t_gate = t_gate.astype(np.float32)
    t_gh = (t_gate @ W_hh.T).astype(np.float32)  # [32, 384]
    t_ih = np.ascontiguousarray(W_ih.T).astype(np.float32)  # [32, 384]
    tbl = np.concatenate([t_ih, t_gh, t_gate], axis=1).astype(np.float32)  # [32, 896]

    cst = np.zeros((128, CW), np.float32)
    cst[:, C_IOTA32 : C_IOTA32 + 32] = np.arange(32, dtype=np.float32)[None, :]
    cst[:, C_BIAS4 + 0] = b_ih[0:128] + b_hh[0:128]
    cst[:, C_BIAS4 + 1] = b_ih[128:256] + b_hh[128:256]
    cst[:, C_BIAS4 + 2] = b_ih[256:384]
    cst[:, C_BIAS4 + 3] = b_hh[256:384]
    cst[:, C_W12 : C_W12 + NZ] = W1.T
    cst[:, C_W12 + NZ : C_W12 + 2 * NZ] = W2.T
    cst[0:NZ, C_B12 + 0] = b1
    cst[0:NZ, C_B12 + 1] = b2
    cst[0:32, C_IOTAP] = np.arange(32, dtype=np.float32)

    adj_bf = np.asarray(adj, np.int32).astype(bf16)  # 0/1 exact
    nt = np.asarray(node_types, np.int32)

    in_maps = []
    for c in range(NCORES):
        sl = slice(c * BL, (c + 1) * BL)
        a_c = adj_bf[sl]  # [BL, 256, 256]
        a_t = np.concatenate([a_c[:, :128, :], a_c[:, 128:, :]], axis=2)
        nt_c = nt[sl].astype(np.float32)  # [BL, 256]
        cst_c = cst.copy()
        cst_c[:, C_NTCOLS : C_NTCOLS + 2 * BL] = (
            nt_c.reshape(BL, 2, 128).transpose(2, 0, 1).reshape(128, 2 * BL))
        nt_bcast = np.ascontiguousarray(
            np.broadcast_to(nt_c.reshape(1, BL * N), (32, BL * N)))
        in_maps.append({
            "a_t": np.ascontiguousarray(a_t).reshape(BL, 128, 512),
            "cst": cst_c,
            "tbl": tbl,
            "nt_bcast": nt_bcast,
        })

    if "nc" not in _CACHE:
        _CACHE["nc"] = _build_nc()
    _CACHE["in_maps"] = in_maps
    res = run_bass_kernel_spmd(_CACHE["nc"], in_maps, core_ids=list(range(NCORES)))

    mu = np.empty((B, NZ), np.float32)
    logvar = np.empty((B, NZ), np.float32)
    for c in range(NCORES):
        o = res.results[c]["out"]  # [NZ, 2*BL]
        mu[c * BL : (c + 1) * BL] = o[:, 0:BL].T
        logvar[c * BL : (c + 1) * BL] = o[:, BL : 2 * BL].T
    return (mu, logvar)


# revision 11
# speedup vs baseline: 145.0720x; 145.0720x over previous
"""DVAE GNN message-passing kernel for 8 Trainium2 NeuronCores.

Math restructuring (the core trick):
  X = one_hot(node_types, 32) is one-hot over only NVT=32 types, so every
  per-type linear layer is a 32-row table lookup, and the predecessor
  aggregation factors through a tiny "predecessor type count" matrix:

     C[b]   = A[b]^T @ X[b]                  [256, 32]  (ints)
     Hpre   = C @ T_gate                     T_gate = sigmoid(Wg^T+bg)*Wm^T  [32,128]
     gi     = X @ W_ih^T + b_ih              table T_ih = W_ih^T             [32,384]
     gh     = Hpre @ W_hh^T + b_hh = C @ T_gh + b_hh,  T_gh = T_gate @ W_hh^T [32,384]

  Per batch the only O(N^2) work is C^T = X^T A (two 128-contraction
  matmuls, exact in bf16 since everything is 0/1 and counts < 256); all
  other matmuls are K=32 table matmuls done in fp32r at full PE rate.
  GRU elementwise + graph-sum are fused DVE/ACT ops on [128, 254] tiles
  in (feature, node) layout; only nodes 1..254 are ever consumed.

Sharding: pure data parallel, 64 batches per core; tables replicated.
"""

import sys
from contextlib import ExitStack

import numpy as np

sys.path.insert(0, "/opt/trn_rl_repo")

import ml_dtypes  # noqa: E402
import concourse.bass as bass  # noqa: E402
import concourse.bacc as bacc  # noqa: E402
import concourse.tile as tile  # noqa: E402
from concourse import mybir  # noqa: E402
from concourse.bass_utils import run_bass_kernel_spmd  # noqa: E402

B, N, NVT, HS, NZ = 512, 256, 32, 128, 64
NCORES = 8
BL = B // NCORES  # 64 batches per core

F32 = mybir.dt.float32
F32R = mybir.dt.float32r
BF16 = mybir.dt.bfloat16
EQ = mybir.AluOpType.is_equal
ADD = mybir.AluOpType.add
SUB = mybir.AluOpType.subtract
MUL = mybir.AluOpType.mult
SIG = mybir.ActivationFunctionType.Sigmoid
TANH = mybir.ActivationFunctionType.Tanh
IDENT = mybir.ActivationFunctionType.Identity

# layout of the packed f32 constant block [128, CW]
C_IOTA32 = 0          # [128, 32]
C_NTCOLS = 32         # [128, 2*BL]
C_BIAS4 = 32 + 2 * BL         # [128, 4]
C_W12 = C_BIAS4 + 4           # [128, 2*NZ]
C_B12 = C_W12 + 2 * NZ        # [64, 2]
C_IOTAP = C_B12 + 2           # [32, 1]
CW = C_IOTAP + 1

_CACHE = {}


def _build_nc(reps=1):
    nc = bacc.Bacc()

    a_d = nc.declare_dram_parameter("a_t", [BL, 128, 512], BF16, False)
    cst_d = nc.declare_dram_parameter("cst", [128, CW], F32, False)
    tbl_d = nc.declare_dram_parameter("tbl", [32, 896], F32R, False)
    ntb_d = nc.declare_dram_parameter("nt_bcast", [32, BL * N], F32, False)
    out_d = nc.declare_dram_parameter("out", [NZ, 2 * BL], F32, True)

    with tile.TileContext(nc) as tc, ExitStack() as ctx:
        cpool = ctx.enter_context(tc.tile_pool(name="const", bufs=1))
        apool = ctx.enter_context(tc.tile_pool(name="a", bufs=3))
        xpool = ctx.enter_context(tc.tile_pool(name="x", bufs=3))
        epool = ctx.enter_context(tc.tile_pool(name="ew", bufs=3))
        ppool = ctx.enter_context(tc.tile_pool(name="ps", bufs=2, space="PSUM"))

        cst = cpool.tile([128, CW], F32, tag="cst")
        nc.sync.dma_start(out=cst, in_=cst_d[:, :])
        tbl = cpool.tile([32, 896], F32R, tag="tbl")
        nc.sync.dma_start(out=tbl, in_=tbl_d[:, :])
        nt_bcast = cpool.tile([32, BL * N], F32, tag="ntb")
        nc.sync.dma_start(out=nt_bcast, in_=ntb_d[:, :])

        iota32 = cst[:, C_IOTA32 : C_IOTA32 + 32]
        nt_cols = cst[:, C_NTCOLS : C_NTCOLS + 2 * BL]
        bias4 = cst[:, C_BIAS4 : C_BIAS4 + 4]
        w12 = cst[:, C_W12 : C_W12 + 2 * NZ]
        b12 = cst[0:NZ, C_B12 : C_B12 + 2]
        iotap = cst[0:32, C_IOTAP : C_IOTAP + 1]
        t_ih = tbl[:, 0:384]
        t_gh = tbl[:, 384:768]
        t_gate = tbl[:, 768:896]

        hg = cpool.tile([128, BL], F32, tag="hg")
        out_sb = cpool.tile([NZ, 2 * BL], F32, tag="out_sb")

        def batch_body(b):
            a_t = apool.tile([128, 512], BF16, tag="a")
            nc.sync.dma_start(out=a_t, in_=a_d[b, :, :])

            # one-hot of this batch's node types, in both layouts
            x01 = xpool.tile([128, 64], BF16, tag="x01")
            nc.gpsimd.tensor_scalar(
                out=x01[:, 0:32], in0=iota32, scalar1=nt_cols[:, 2 * b : 2 * b + 1],
                scalar2=None, op0=EQ)
            nc.gpsimd.tensor_scalar(
                out=x01[:, 32:64], in0=iota32, scalar1=nt_cols[:, 2 * b + 1 : 2 * b + 2],
                scalar2=None, op0=EQ)
            xt = xpool.tile([32, N], F32R, tag="xt")
            nc.gpsimd.tensor_scalar(
                out=xt, in0=nt_bcast[:, b * N : (b + 1) * N], scalar1=iotap,
                scalar2=None, op0=EQ)

            # C^T[t, v] = sum_u X[u, t] A[u, v]   (exact integer counts)
            c_ps = ppool.tile([32, N], F32, tag="c")
            nc.tensor.matmul(out=c_ps, lhsT=x01[:, 0:32], rhs=a_t[:, 0:256],
                             start=True, stop=False)
            nc.tensor.matmul(out=c_ps, lhsT=x01[:, 32:64], rhs=a_t[:, 256:512],
                             start=False, stop=True)
            c_sb = xpool.tile([32, N], F32R, tag="csb")
            nc.vector.tensor_copy(out=c_sb, in_=c_ps)

            # gate pre-activations: [gate_chunk, node] layout
            rz_ps = ppool.tile([128, 512], F32, tag="rz")
            nc.tensor.matmul(out=rz_ps[:, 0:256], lhsT=t_ih[:, 0:128],
                             rhs=xt, start=True, stop=False)
            nc.tensor.matmul(out=rz_ps[:, 0:256], lhsT=t_gh[:, 0:128],
                             rhs=c_sb, start=False, stop=True)
            nc.tensor.matmul(out=rz_ps[:, 256:512], lhsT=t_ih[:, 128:256],
                             rhs=xt, start=True, stop=False)
            nc.tensor.matmul(out=rz_ps[:, 256:512], lhsT=t_gh[:, 128:256],
                             rhs=c_sb, start=False, stop=True)
            n_ps = ppool.tile([128, 512], F32, tag="n")
            nc.tensor.matmul(out=n_ps[:, 0:256], lhsT=t_ih[:, 256:384],
                             rhs=xt)
            nc.tensor.matmul(out=n_ps[:, 256:512], lhsT=t_gh[:, 256:384],
                             rhs=c_sb)
            hp_ps = ppool.tile([128, N], F32, tag="hp")
            nc.tensor.matmul(out=hp_ps, lhsT=t_gate, rhs=c_sb)

            # GRU + graph-sum on nodes 1..254 only
            r_t = epool.tile([128, 254], F32, tag="r")
            nc.scalar.activation(out=r_t, in_=rz_ps[:, 1:255], func=SIG,
                                 bias=bias4[:, 0:1], scale=1.0)
            z_t = epool.tile([128, 254], F32, tag="z")
            nc.scalar.activation(out=z_t, in_=rz_ps[:, 257:511], func=SIG,
                                 bias=bias4[:, 1:2], scale=1.0)
            t_t = epool.tile([128, 254], F32, tag="t")
            nc.vector.scalar_tensor_tensor(
                out=t_t, in0=n_ps[:, 257:511], scalar=bias4[:, 3:4], in1=r_t,
                op0=ADD, op1=MUL)
            s_t = epool.tile([128, 254], F32, tag="s")
            nc.vector.tensor_tensor(out=s_t, in0=t_t, in1=n_ps[:, 1:255], op=ADD)
            nn_t = epool.tile([128, 254], F32, tag="nn")
            nc.scalar.activation(out=nn_t, in_=s_t, func=TANH,
                                 bias=bias4[:, 2:3], scale=1.0)
            d_t = epool.tile([128, 254], F32, tag="d")
            nc.vector.tensor_tensor(out=d_t, in0=hp_ps[:, 1:255], in1=nn_t, op=SUB)
            e_t = epool.tile([128, 254], F32, tag="e")
            nc.vector.tensor_tensor(out=e_t, in0=z_t, in1=d_t, op=MUL)
            f_t = epool.tile([128, 254], F32, tag="f")
            nc.vector.scalar_tensor_tensor(
                out=f_t, in0=nn_t, scalar=0.0, in1=e_t, op0=ADD, op1=ADD,
                accum_out=hg[:, b : b + 1])

        if reps == 1:
            for b in range(BL):
                batch_body(b)
        else:
            with tc.For_i(0, reps, 1, hint_engines=tuple(nc.engines)):
                for b in range(BL):
                    batch_body(b)

        # mu^T / logvar^T: [z, batch] = W^T.T @ Hg_all
        mu_ps = ppool.tile([NZ, BL], F32, tag="c")
        nc.tensor.matmul(out=mu_ps, lhsT=w12[:, 0:NZ], rhs=hg)
        nc.scalar.activation(out=out_sb[:, 0:BL], in_=mu_ps, func=IDENT,
                             bias=b12[:, 0:1], scale=1.0)
        lv_ps = ppool.tile([NZ, BL], F32, tag="c")
        nc.tensor.matmul(out=lv_ps, lhsT=w12[:, NZ : 2 * NZ], rhs=hg)
        nc.scalar.activation(out=out_sb[:, BL : 2 * BL], in_=lv_ps, func=IDENT,
                             bias=b12[:, 1:2], scale=1.0)
        nc.sync.dma_start(out=out_d[:, :], in_=out_sb)

    nc.finalize()
    return nc


def kernel(node_types, adj, W_ih, W_hh, b_ih, b_hh, Wg, bg, Wm, W1, b1, W2, b2):
    bf16 = ml_dtypes.bfloat16

    # host-folded tables (weight preprocessing only; all O(B) work on device)
    t_gate = (1.0 / (1.0 + np.exp(-(Wg.T.astype(np.float64) + bg[None, :])))) * Wm.T
    t_gate = t_gate.astype(np.float32)
    t_gh = (t_gate @ W_hh.T).astype(np.float32)  # [32, 384]
    t_ih = np.ascontiguousarray(W_ih.T).astype(np.float32)  # [32, 384]
    tbl = np.concatenate([t_ih, t_gh, t_gate], axis=1).astype(np.float32)  # [32, 896]

    cst = np.zeros((128, CW), np.float32)
    cst[:, C_IOTA32 : C_IOTA32 + 32] = np.arange(32, dtype=np.float32)[None, :]
    cst[:, C_BIAS4 + 0] = b_ih[0:128] + b_hh[0:128]
    cst[:, C_BIAS4 + 1] = b_ih[128:256] + b_hh[128:256]
    cst[:, C_BIAS4 + 2] = b_ih[256:384]
    cst[:, C_BIAS4 + 3] = b_hh[256:384]
    cst[:, C_W12 : C_W12 + NZ] = W1.T
    cst[:, C_W12 + NZ : C_W12 + 2 * NZ] = W2.T
    cst[0:NZ, C_B12 + 0] = b1
    cst[0:NZ, C_B12 + 1] = b2
    cst[0:32, C_IOTAP] = np.arange(32, dtype=np.float32)

    adj_bf = np.asarray(adj, np.int32).astype(bf16)  # 0/1 exact
    nt = np.asarray(node_types, np.int32)

    in_maps = []
    for c in range(NCORES):
        sl = slice(c * BL, (c + 1) * BL)
        a_c = adj_bf[sl]  # [BL, 256, 256]
        a_t = np.concatenate([a_c[:, :128, :], a_c[:, 128:, :]], axis=2)
        nt_c = nt[sl].astype(np.float32)  # [BL, 256]
        cst_c = cst.copy()
        cst_c[:, C_NTCOLS : C_NTCOLS + 2 * BL] = (
            nt_c.reshape(BL, 2, 128).transpose(2, 0, 1).reshape(128, 2 * BL))
        nt_bcast = np.ascontiguousarray(
            np.broadcast_to(nt_c.reshape(1, BL * N), (32, BL * N)))
        in_maps.append({
            "a_t": np.ascontiguousarray(a_t).reshape(BL, 128, 512),
            "cst": cst_c,
            "tbl": tbl,
            "nt_bcast": nt_bcast,
        })

    if "nc" not in _CACHE:
        _CACHE["nc"] = _build_nc()
    _CACHE["in_maps"] = in_maps
    res = run_bass_kernel_spmd(_CACHE["nc"], in_maps, core_ids=list(range(NCORES)))

    mu = np.empty((B, NZ), np.float32)
    logvar = np.empty((B, NZ), np.float32)
    for c in range(NCORES):
        o = res.results[c]["out"]  # [NZ, 2*BL]
        mu[c * BL : (c + 1) * BL] = o[:, 0:BL].T
        logvar[c * BL : (c + 1) * BL] = o[:, BL : 2 * BL].T
    return (mu, logvar)
